# revision 33
# baseline (speedup 1.0000x reference)
import sys, os
sys.path.insert(0, "/opt/trn_rl_repo")
import numpy as np
import ml_dtypes
from contextlib import ExitStack

import concourse.bass as bass
import concourse.bacc as bacc
import concourse.tile as tile
from concourse import mybir
from concourse.bass_utils import run_bass_kernel_spmd

f32 = mybir.dt.float32
bf16 = mybir.dt.bfloat16
u32 = mybir.dt.uint32
AF = mybir.ActivationFunctionType
ALU = mybir.AluOpType
AX = mybir.AxisListType
bfnp = ml_dtypes.bfloat16

B, L, D, K = 16, 4096, 1024, 5
NCORES = 8
BPC = B // NCORES          # examples per core
DC = D // 128              # 8 contraction chunks
NJ = L // 512              # 8 moving chunks of 512
NCAND = 8                  # top-8 candidates, exact top-5 refinement
SCALE = 1.0 / float(np.sqrt(D))

_NC_CACHE = {}
LAST = {}


def _build_nc():
    if "nc" in _NC_CACHE:
        return _NC_CACHE["nc"]
    nc = bacc.Bacc("TRN2", target_bir_lowering=False, debug=False,
                   num_devices=NCORES)
    dI = lambda n, s, dt=bf16: nc.dram_tensor(n, s, dt, kind="ExternalInput").ap()
    ht_d = dI("ht", [BPC, DC, 128, L])          # H^T hi, chunked by d
    nat_d = dI("nat", [BPC, L, D])              # H hi, natural layout
    hrow_d = dI("hrow", [BPC, L, D], f32)       # raw fp32 H for row gather
    wq_d = dI("wq", [D, D]); wkt_d = dI("wkt", [D, D])
    wv_d = dI("wv", [D, D]); wct_d = dI("wct", [D, D])
    wsb_d = dI("wsb", [DC, 128, 1])             # w_start hi, chunked
    wsf8_d = dI("wsf8", [NCAND, D], f32)        # w_start fp32, replicated rows
    i8_d = dI("i8", [NCAND, NCAND], f32)
    ones8_d = dI("ones8", [NCAND, 1])
    i32_d = dI("i32", [32, 32])
    sl_d = nc.dram_tensor("sl", [BPC, L], f32, kind="ExternalOutput").ap()
    el_d = nc.dram_tensor("el", [BPC, L], f32, kind="ExternalOutput").ap()

    with tile.TileContext(nc) as tc, ExitStack() as ctx:
        res = ctx.enter_context(tc.tile_pool(name="res", bufs=1))
        wstg = ctx.enter_context(tc.tile_pool(name="wstg", bufs=4))
        sm = ctx.enter_context(tc.tile_pool(name="sm", bufs=1))
        pbig = ctx.enter_context(tc.tile_pool(name="pbig", bufs=5, space="PSUM"))
        psm = ctx.enter_context(tc.tile_pool(name="psm", bufs=2, space="PSUM"))

        # ---- resident loads
        ht_sb = []
        for b in range(BPC):
            htt = res.tile([128, DC, L], bf16, tag=f"ht{b}", name=f"ht{b}")
            ht_sb.append(htt)
            for dc in range(DC):
                nc.gpsimd.dma_start(htt[:, dc, :], ht_d[b, dc])
        wsb = res.tile([128, DC, 1], bf16)
        for dc in range(DC):
            nc.sync.dma_start(wsb[:, dc, :], wsb_d[dc])
        wsf8 = res.tile([NCAND, D], f32); nc.sync.dma_start(wsf8[:], wsf8_d[:])
        i8 = res.tile([NCAND, NCAND], f32); nc.sync.dma_start(i8[:], i8_d[:])
        ones8 = res.tile([NCAND, 1], bf16); nc.sync.dma_start(ones8[:], ones8_d[:])
        i32 = res.tile([32, 32], bf16); nc.sync.dma_start(i32[:], i32_d[:])
        natp = ctx.enter_context(tc.tile_pool(name="natp", bufs=4))

        # ---- S1: start logits, flipped orientation (ws stationary, ht moving)
        fL_sb = []
        for b in range(BPC):
            fL = sm.tile([1, L], bf16, tag=f"sct{b}", name="fL")
            fL_sb.append(fL)
            for j in range(NJ):
                psL = pbig.tile([16, 512], f32, tag="mm", name="psL")
                for dc in range(DC):
                    nc.tensor.matmul(psL[0:1, :], wsb[:, dc, :],
                                     ht_sb[b][:, dc, j * 512:(j + 1) * 512],
                                     start=(dc == 0), stop=(dc == DC - 1))
                ltmp = sm.tile([1, 512], f32, tag="ltmp", bufs=2, name="ltmp")
                nc.scalar.copy(ltmp[:], psL[0:1, :])
                nc.scalar.copy(fL[0:1, j * 512:(j + 1) * 512], psL[0:1, :])
                nc.sync.dma_start(sl_d[b:b + 1, j * 512:(j + 1) * 512], ltmp[:])

        # ---- S2: top-8 candidates + exact fp32 refinement -> masked weights,
        #      then S3: transpose gathered rows into srhs (per example)
        srhs = sm.tile([128, DC, BPC, 2, NCAND], bf16, tag="srhs", name="srhs")
        sr_hf = sm.tile([128, DC, NCAND], f32, tag="sr_hf", name="sr_hf")
        sr_lf = sm.tile([128, DC, NCAND], f32, tag="sr_lf", name="sr_lf")
        # ---- S4 (per example): Q^T then P^T chains, then S5 scores
        def wchain(w_d, rhs, b, tag):
            ps4 = psm.tile([128, DC, 2, NCAND], f32, tag="sm", name="ps4")
            for dci in range(DC):
                wt = wstg.tile([128, D], bf16, tag="wt", name="wt")
                nc.gpsimd.dma_start(wt[:], w_d[dci * 128:(dci + 1) * 128, :])
                for dco in range(DC):
                    # one global start per psum tile: a later start=True would
                    # clobber sibling regions' accumulation state in the bank
                    nc.tensor.matmul(ps4[:, dco, :, :],
                                     wt[:, dco * 128:(dco + 1) * 128],
                                     rhs[:, dci, b, :, :],
                                     start=(dci == 0 and dco == 0),
                                     stop=(dci == DC - 1),
                                     skip_group_check=True)
            qf = sm.tile([128, DC, NCAND], f32, tag=tag + "f", name=tag + "f")
            nc.vector.tensor_copy(qf[:], ps4[:, :, 0, :])
            nc.vector.tensor_add(qf[:], qf[:], ps4[:, :, 1, :])
            pair = sm.tile([128, DC, 1, 2, NCAND], bf16, tag=tag, name=tag)
            nc.vector.tensor_copy(pair[:, :, 0, 0, :], qf[:])
            hf = sm.tile([128, DC, NCAND], f32, tag=tag + "h", name=tag + "h")
            nc.vector.tensor_copy(hf[:], pair[:, :, 0, 0, :])
            nc.vector.tensor_sub(qf[:], qf[:], hf[:])
            nc.vector.tensor_copy(pair[:, :, 0, 1, :], qf[:])
            return pair

        sct_sb, mxc_sb = [], []

        def s4s5(b):
            qpair = wchain(wq_d, srhs, b, f"qp{b}")
            ppair = wchain(wkt_d, qpair, 0, f"pp{b}")
            sct = sm.tile([NCAND, L], bf16, tag=f"sct{b}", name=f"sct{b}")
            sct_sb.append(sct)
            mxc = sm.tile([NCAND, NJ], f32, tag=f"mxc{b}", name=f"mxc{b}")
            mxc_sb.append(mxc)
            for j in range(NJ):
                ps5 = pbig.tile([16, 512], f32, tag="mm", name="ps5")
                for dc in range(DC):
                    nc.tensor.matmul(ps5[:], ppair[:, dc, 0, :, :],
                                     ht_sb[b][:, dc, j * 512:(j + 1) * 512],
                                     start=(dc == 0), stop=(dc == DC - 1))
                # fold hi+lo rows: engines cannot cross partition bases, so
                # stage via scalar copy + DMA partition move, then DVE add
                cp5 = sm.tile([16, 512], f32, tag="cp5", bufs=2, name="cp5")
                nc.scalar.copy(cp5[:], ps5[:])
                cp5b = sm.tile([NCAND, 512], f32, tag="cp5b", bufs=2, name="cp5b")
                nc.sync.dma_start(cp5b[:], cp5[NCAND:16, :])
                nc.vector.tensor_add(sct[:, j * 512:(j + 1) * 512],
                                     cp5[0:NCAND, :], cp5b[:])
                nc.vector.tensor_reduce(mxc[:, j:j + 1],
                                        sct[:, j * 512:(j + 1) * 512],
                                        AX.X, ALU.max)


        hrow_flat = hrow_d.rearrange("b l d -> (b l) d")
        wn8_sb = []
        for b in range(BPC):
            t8v = sm.tile([1, 8], bf16, tag=f"t8v{b}", name="t8v")
            t8p = sm.tile([1, 8], u32, tag=f"t8p{b}", name="t8p")
            nc.vector.max(t8v[:], fL_sb[b][:])
            nc.vector.max_index(t8p[:], t8v[:], fL_sb[b][:])
            t8pf = sm.tile([1, 8], f32, tag=f"t8pf{b}", name="t8pf")
            nc.vector.tensor_copy(t8pf[:], t8p[:])
            nc.vector.tensor_scalar_add(t8pf[:], t8pf[:], float(b * L))
            t8pi = sm.tile([1, 8], u32, tag=f"t8pi{b}", name="t8pi")
            nc.vector.tensor_copy(t8pi[:], t8pf[:])
            idx8 = sm.tile([NCAND, 1], u32, tag=f"idx8{b}", name="idx8")
            nc.sync.dma_start(idx8[:, 0:1], t8pi[0:1, :])
            rows = sm.tile([NCAND, D], f32, tag="rows", name="rows")
            nc.gpsimd.indirect_dma_start(
                out=rows[:], out_offset=None, in_=hrow_flat,
                in_offset=bass.IndirectOffsetOnAxis(ap=idx8[:, 0:1], axis=0))
            # exact fp32 logits for the 8 candidates (f32 products so the
            # reduce is fp32-exact; tensor_tensor_reduce is avoided — it
            # crashes the device on this runtime)
            prod = sm.tile([NCAND, D], f32, tag="ek", name="prod")
            e8 = sm.tile([NCAND, 1], f32, tag=f"e8{b}", name="e8")
            nc.vector.tensor_mul(prod[:], rows[:], wsf8[:])
            nc.vector.tensor_reduce(e8[:], prod[:], AX.X, ALU.add)
            e8r = sm.tile([1, 8], f32, tag=f"e8r{b}", name="e8r")
            nc.sync.dma_start(e8r[0:1, :], e8[:])
            s8 = sm.tile([1, 8], f32, tag=f"s8{b}", name="s8")
            nc.vector.max(s8[:], e8r[:])
            thr = sm.tile([1, 1], f32, tag=f"thr{b}", name="thr")
            nc.vector.tensor_add(thr[:], s8[0:1, K - 1:K], s8[0:1, K:K + 1])
            nc.vector.tensor_scalar_mul(thr[:], thr[:], 0.5)
            msk = sm.tile([1, 8], f32, tag=f"msk{b}", name="msk")
            nc.vector.tensor_scalar(msk[:], e8r[:], thr[:], None, ALU.is_gt)
            negmx = sm.tile([1, 1], f32, tag=f"negmx{b}", name="negmx")
            nc.vector.tensor_scalar_mul(negmx[:], s8[0:1, 0:1], -1.0)
            ew = sm.tile([1, 8], f32, tag=f"ew{b}", name="ew")
            nc.scalar.activation(ew[:], e8r[:], AF.Exp, bias=negmx[:], scale=1.0)
            w8m = sm.tile([1, 8], f32, tag=f"w8m{b}", name="w8m")
            nc.vector.tensor_mul(w8m[:], ew[:], msk[:])
            sw = sm.tile([1, 1], f32, tag=f"sw{b}", name="sw")
            nc.vector.tensor_reduce(sw[:], w8m[:], AX.X, ALU.add)
            rsw = sm.tile([1, 1], f32, tag=f"rsw{b}", name="rsw")
            nc.vector.reciprocal(rsw[:], sw[:])
            wn = sm.tile([1, 8], f32, tag=f"wn{b}", name="wn")
            nc.vector.tensor_scalar_mul(wn[:], w8m[:], rsw[:])
            wn8 = sm.tile([NCAND, 1], f32, tag=f"wn8_{b}", name=f"wn8_{b}")
            wn8_sb.append(wn8)
            nc.sync.dma_start(wn8[:, 0:1], wn[0:1, :])

            # S3 for this example: PE transpose of the gathered fp32 rows
            psr = psm.tile([128, DC, NCAND], f32, tag="sm", name="psr")
            for dc in range(DC):
                nc.tensor.matmul(psr[:, dc, :],
                                 rows[:, dc * 128:(dc + 1) * 128], i8[:],
                                 is_transpose=True, start=True, stop=True,
                                 skip_group_check=True)
            nc.vector.tensor_copy(srhs[:, :, b, 0, :], psr[:])
            nc.vector.tensor_copy(sr_hf[:], srhs[:, :, b, 0, :])
            nc.vector.tensor_sub(sr_lf[:], psr[:], sr_hf[:])
            nc.vector.tensor_copy(srhs[:, :, b, 1, :], sr_lf[:])
            s4s5(b)

        # ---- softmax + m broadcast + S6 per example (shared big scratch)
        amix = sm.tile([128, DC, BPC], f32, tag="amix", name="amix")
        for b in range(BPC):
            mx8 = sm.tile([NCAND, 1], f32, tag="mx8", name="mx8")
            nc.vector.tensor_reduce(mx8[:], mxc_sb[b][:], AX.X, ALU.max)
            nbias = sm.tile([NCAND, 1], f32, tag="nbias", name="nbias")
            nc.vector.tensor_scalar_mul(nbias[:], mx8[:], -SCALE)
            ek = sm.tile([NCAND, L], bf16, tag="ek", name="ek")
            z8 = sm.tile([NCAND, 1], f32, tag="z8", name="z8")
            nc.scalar.activation(ek[:], sct_sb[b][:], AF.Exp, bias=nbias[:],
                                 scale=SCALE, accum_out=z8[:])
            rz8 = sm.tile([NCAND, 1], f32, tag="rz8", name="rz8")
            nc.vector.reciprocal(rz8[:], z8[:])
            c8 = sm.tile([NCAND, 1], f32, tag="c8", name="c8")
            nc.vector.tensor_mul(c8[:], wn8_sb[b][:], rz8[:])
            nc.vector.tensor_scalar_mul(ek[:], ek[:], c8[:])   # ek *= c8
            # m as [32, 128] (nat-layout rows), then transpose to [128, 32]
            mt32 = sm.tile([32, 128], bf16, tag="mt32", name="mt32")
            for j in range(NJ):
                pm = pbig.tile([16, 512], f32, tag="mm", name="pm")
                nc.tensor.matmul(pm[0:1, :], ones8[:],
                                 ek[:, j * 512:(j + 1) * 512],
                                 start=True, stop=True)
                m1 = sm.tile([1, 512], bf16, tag="m1", bufs=2, name="m1")
                nc.scalar.copy(m1[:], pm[0:1, :])
                nc.sync.dma_start(mt32[4 * j:4 * j + 4, :], m1[:])
            pt = psm.tile([128, 32], bf16, tag="sm", name="pt")
            nc.tensor.matmul(pt[:], mt32[:], i32[:], is_transpose=True,
                             start=True, stop=True)
            mt = sm.tile([128, 32], bf16, tag="mt", name="mt")
            nc.vector.tensor_copy(mt[:], pt[:])
            # S6: a_mix = sum_l H[l, d] * m[l] on PE, streaming natural H
            ps6 = psm.tile([128, DC, 1], f32, tag="sm", name="ps6")
            NLC = L // 128
            for lc2 in range(NLC // 2):
                nat = natp.tile([128, 2, D], bf16, tag="nat", name="nat")
                eng = nc.gpsimd if lc2 % 2 == 0 else nc.sync
                eng.dma_start(
                    nat[:],
                    nat_d[b, lc2 * 256:(lc2 + 1) * 256, :]
                    .rearrange("(i p) d -> p i d", p=128))
                for i in range(2):
                    lc = lc2 * 2 + i
                    for dc in range(DC):
                        nc.tensor.matmul(ps6[:, dc, :],
                                         nat[:, i, dc * 128:(dc + 1) * 128],
                                         mt[:, lc:lc + 1],
                                         start=(lc == 0 and dc == 0),
                                         stop=(lc == NLC - 1),
                                         skip_group_check=True)
            nc.scalar.copy(amix[:, :, b:b + 1], ps6[:])

        # ---- split helper [128, DC, BPC] f32 -> [128, 2, DC, BPC] bf16
        def split2(src, tag):
            pair = sm.tile([128, DC, 2, BPC], bf16, tag=tag, name=tag)
            nc.vector.tensor_copy(pair[:, :, 0, :], src[:])
            hf = sm.tile([128, DC, BPC], f32, tag=tag + "h", name=tag + "h")
            nc.vector.tensor_copy(hf[:], pair[:, :, 0, :])
            nc.vector.tensor_sub(hf[:], src[:], hf[:])
            nc.vector.tensor_copy(pair[:, :, 1, :], hf[:])
            return pair

        arhs = split2(amix, "arhs")

        # ---- S7: c_mix (wv), g (wct), shared weight streams for both examples
        def wchain2(w_d, rhs, tag):
            ps7 = psm.tile([128, DC, 2, BPC], f32, tag="sm", name="ps7")
            for dci in range(DC):
                wt = wstg.tile([128, D], bf16, tag="wt", name="wt")
                nc.gpsimd.dma_start(wt[:], w_d[dci * 128:(dci + 1) * 128, :])
                for dco in range(DC):
                    nc.tensor.matmul(ps7[:, dco, :, :],
                                     wt[:, dco * 128:(dco + 1) * 128],
                                     rhs[:, dci, :, :],
                                     start=(dci == 0 and dco == 0),
                                     stop=(dci == DC - 1),
                                     skip_group_check=True)
            outf = sm.tile([128, DC, BPC], f32, tag=tag, name=tag)
            nc.vector.tensor_copy(outf[:], ps7[:, :, 0, :])
            nc.vector.tensor_add(outf[:], outf[:], ps7[:, :, 1, :])
            return outf

        cmix = wchain2(wv_d, arhs, "cm")
        crhs = split2(cmix, "crhs")
        g_f = wchain2(wct_d, crhs, "gg")
        gs = sm.tile([128, DC, BPC], f32, tag="gs", name="gs")
        nc.vector.tensor_scalar_mul(gs[:], g_f[:], SCALE)
        grhs = split2(gs, "grhs")

        # ---- S8: end logits from resident ht
        for b in range(BPC):
            for j in range(NJ):
                ps8 = pbig.tile([16, 512], f32, tag="mm", name="ps8")
                for dc in range(DC):
                    nc.tensor.matmul(ps8[0:2, :], grhs[:, dc, :, b],
                                     ht_sb[b][:, dc, j * 512:(j + 1) * 512],
                                     start=(dc == 0), stop=(dc == DC - 1))
                cp8 = sm.tile([2, 512], f32, tag="cp5", bufs=2, name="cp8")
                nc.scalar.copy(cp8[:], ps8[0:2, :])
                cp8b = sm.tile([1, 512], f32, tag="cp5b", bufs=2, name="cp8b")
                nc.sync.dma_start(cp8b[:], cp8[1:2, :])
                etmp = sm.tile([1, 512], f32, tag="ltmp", bufs=2, name="etmp")
                nc.vector.tensor_add(etmp[:], cp8[0:1, :], cp8b[:])
                nc.sync.dma_start(el_d[b:b + 1, j * 512:(j + 1) * 512], etmp[:])

    if os.environ.get("KERNEL_BUILD_INFO"):
        print(f"[kernel] sbuf remaining: {nc.sbuf_bytes_remaining} bytes")
    nc.compile()
    _NC_CACHE["nc"] = nc
    return nc


def _np_reference(H, attention_mask, w_start, b_start, w_q, b_q, w_k, b_k,
                  w_v, b_v, w_cmp, b_cmp):
    NEG = -1e9
    H = H.astype(np.float32)
    pad = attention_mask == 0
    sl = (H @ w_start + b_start)[..., 0]
    sl = np.where(pad, NEG, sl)
    x = sl - sl.max(-1, keepdims=True)
    e = np.exp(x); sp = e / e.sum(-1, keepdims=True)
    idx = np.argsort(-sp, axis=-1, kind="stable")[:, :K]
    tp = np.take_along_axis(sp, idx, axis=1)
    sr = np.take_along_axis(H, idx[..., None], axis=1)
    Q = sr @ w_q + b_q
    K_ = H @ w_k + b_k
    V = H @ w_v + b_v
    sc = np.einsum('bkd,bld->bkl', Q, K_) * SCALE
    sc = np.where(pad[:, None, :], NEG, sc)
    sc = sc - sc.max(-1, keepdims=True)
    a = np.exp(sc); a = a / a.sum(-1, keepdims=True)
    ctx_ = np.einsum('bkl,bld->bkd', a, V)
    tcmp = H @ w_cmp + b_cmp
    es = np.einsum('bkd,bld->bkl', ctx_, tcmp) * SCALE
    es = np.where(pad[:, None, :], NEG, es)
    w = tp / (tp.sum(-1, keepdims=True) + 1e-9)
    el = np.einsum('bk,bkl->bl', w, es)
    el = np.where(pad, NEG, el)
    return sl, el


def kernel(**inputs):
    H = np.asarray(inputs["H"], np.float32)
    mask = np.asarray(inputs["attention_mask"])
    b_start = np.asarray(inputs["b_start"], np.float32)
    biases_zero = all(np.all(np.asarray(inputs[n]) == 0)
                      for n in ["b_q", "b_k", "b_v", "b_cmp"])
    if not bool((mask == 1).all()) or not biases_zero:
        sl, el = _np_reference(**{k: np.asarray(v) for k, v in inputs.items()})
        return np.asarray(sl, np.float32), np.asarray(el, np.float32)

    w_start = np.asarray(inputs["w_start"], np.float32)
    w_q = np.asarray(inputs["w_q"], np.float32)
    w_k = np.asarray(inputs["w_k"], np.float32)
    w_v = np.asarray(inputs["w_v"], np.float32)
    w_cmp = np.asarray(inputs["w_cmp"], np.float32)

    hi = H.astype(bfnp)
    ht = np.ascontiguousarray(hi.transpose(0, 2, 1)).reshape(B, DC, 128, L)
    wsb = w_start[:, 0].astype(bfnp).reshape(DC, 128, 1)
    wsf8 = np.ascontiguousarray(
        np.broadcast_to(w_start[:, 0], (NCAND, D))).astype(np.float32)

    nc = _build_nc()
    in_maps = []
    for c in range(NCORES):
        s = slice(c * BPC, (c + 1) * BPC)
        in_maps.append({
            "ht": ht[s], "hrow": H[s], "nat": hi[s],
            "wq": w_q.astype(bfnp),
            "wkt": np.ascontiguousarray(w_k.T).astype(bfnp),
            "wv": w_v.astype(bfnp),
            "wct": np.ascontiguousarray(w_cmp.T).astype(bfnp),
            "wsb": wsb, "wsf8": wsf8,
            "i8": np.eye(NCAND, dtype=np.float32),
            "ones8": np.ones((NCAND, 1), bfnp),
            "i32": np.eye(32, dtype=np.float32).astype(bfnp),
        })
    import time as _time
    _t0 = _time.time()
    kw = {}
    if os.environ.get("KERNEL_PROFILE"):
        kw = dict(trace=True,
                  tmpdir=os.environ.get("KERNEL_PROFILE_DIR") or None,
                  trace_cores=[int(x) for x in
                               os.environ.get("KERNEL_TRACE_CORES", "0").split(",")])
    res = run_bass_kernel_spmd(nc, in_maps, core_ids=list(range(NCORES)), **kw)
    LAST["res"] = res
    if os.environ.get("KERNEL_TIME"):
        print(f"[kernel] device dispatch+exec wall: {_time.time() - _t0:.3f}s")
    sl = np.concatenate([r["sl"] for r in res.results], 0) + b_start[0]
    el = np.concatenate([r["el"] for r in res.results], 0)
    return sl.astype(np.float32), el.astype(np.float32)


# revision 34
# speedup vs baseline: 1.0916x; 1.0916x over previous
import sys, os
sys.path.insert(0, "/opt/trn_rl_repo")
import numpy as np
import ml_dtypes
from contextlib import ExitStack

import concourse.bass as bass
import concourse.bacc as bacc
import concourse.tile as tile
from concourse import mybir
from concourse.bass_utils import run_bass_kernel_spmd

f32 = mybir.dt.float32
bf16 = mybir.dt.bfloat16
u32 = mybir.dt.uint32
AF = mybir.ActivationFunctionType
ALU = mybir.AluOpType
AX = mybir.AxisListType
bfnp = ml_dtypes.bfloat16

B, L, D, K = 16, 4096, 1024, 5
NCORES = 8
BPC = B // NCORES          # examples per core
DC = D // 128              # 8 contraction chunks
NJ = L // 512              # 8 moving chunks of 512
NCAND = 8                  # top-8 candidates, exact top-5 refinement
SCALE = 1.0 / float(np.sqrt(D))

_NC_CACHE = {}
LAST = {}


def _build_nc():
    if "nc" in _NC_CACHE:
        return _NC_CACHE["nc"]
    nc = bacc.Bacc("TRN2", target_bir_lowering=False, debug=False,
                   num_devices=NCORES)
    dI = lambda n, s, dt=bf16: nc.dram_tensor(n, s, dt, kind="ExternalInput").ap()
    ht_d = dI("ht", [BPC, DC, 128, L])          # H^T hi, chunked by d
    nat_d = dI("nat", [BPC, L, D])              # H hi, natural layout
    hrow_d = dI("hrow", [BPC, L, D], f32)       # raw fp32 H for row gather
    wq_d = dI("wq", [D, D]); wkt_d = dI("wkt", [D, D])
    wv_d = dI("wv", [D, D]); wct_d = dI("wct", [D, D])
    wsb_d = dI("wsb", [DC, 128, 1])             # w_start hi, chunked
    wsf8_d = dI("wsf8", [NCAND, D], f32)        # w_start fp32, replicated rows
    i8_d = dI("i8", [NCAND, NCAND], f32)
    ones8_d = dI("ones8", [NCAND, 1])
    i32_d = dI("i32", [32, 32])
    sl_d = nc.dram_tensor("sl", [BPC, L], f32, kind="ExternalOutput").ap()
    el_d = nc.dram_tensor("el", [BPC, L], f32, kind="ExternalOutput").ap()

    with tile.TileContext(nc) as tc, ExitStack() as ctx:
        res = ctx.enter_context(tc.tile_pool(name="res", bufs=1))
        wstg = ctx.enter_context(tc.tile_pool(name="wstg", bufs=4))
        sm = ctx.enter_context(tc.tile_pool(name="sm", bufs=1))
        pbig = ctx.enter_context(tc.tile_pool(name="pbig", bufs=5, space="PSUM"))
        psm = ctx.enter_context(tc.tile_pool(name="psm", bufs=2, space="PSUM"))

        # ---- resident loads
        ht_sb = []
        for b in range(BPC):
            htt = res.tile([128, DC, L], bf16, tag=f"ht{b}", name=f"ht{b}")
            ht_sb.append(htt)
            for dc in range(DC):
                nc.gpsimd.dma_start(htt[:, dc, :], ht_d[b, dc])
        wsb = res.tile([128, DC, 1], bf16)
        for dc in range(DC):
            nc.sync.dma_start(wsb[:, dc, :], wsb_d[dc])
        wsf8 = res.tile([NCAND, D], f32); nc.sync.dma_start(wsf8[:], wsf8_d[:])
        i8 = res.tile([NCAND, NCAND], f32); nc.sync.dma_start(i8[:], i8_d[:])
        ones8 = res.tile([NCAND, 1], bf16); nc.sync.dma_start(ones8[:], ones8_d[:])
        i32 = res.tile([32, 32], bf16); nc.sync.dma_start(i32[:], i32_d[:])
        natp = ctx.enter_context(tc.tile_pool(name="natp", bufs=4))

        # ---- S1: start logits, flipped orientation (ws stationary, ht moving)
        fL_sb = []
        for b in range(BPC):
            fL = sm.tile([1, L], bf16, tag=f"sct{b}", name="fL")
            fL_sb.append(fL)
            for j in range(NJ):
                psL = pbig.tile([16, 512], f32, tag="mm", name="psL")
                for dc in range(DC):
                    nc.tensor.matmul(psL[0:1, :], wsb[:, dc, :],
                                     ht_sb[b][:, dc, j * 512:(j + 1) * 512],
                                     start=(dc == 0), stop=(dc == DC - 1))
                ltmp = sm.tile([1, 512], f32, tag="ltmp", bufs=2, name="ltmp")
                nc.scalar.copy(ltmp[:], psL[0:1, :])
                nc.scalar.copy(fL[0:1, j * 512:(j + 1) * 512], psL[0:1, :])
                nc.sync.dma_start(sl_d[b:b + 1, j * 512:(j + 1) * 512], ltmp[:])

        # ---- S2: top-8 candidates + exact fp32 refinement -> masked weights,
        #      then S3: transpose gathered rows into srhs (per example)
        srhs = sm.tile([128, DC, BPC, 2, NCAND], bf16, tag="srhs", name="srhs")
        sr_hf = sm.tile([128, DC, NCAND], f32, tag="sr_hf", name="sr_hf")
        sr_lf = sm.tile([128, DC, NCAND], f32, tag="sr_lf", name="sr_lf")
        # ---- S4: Q^T then P^T chains (weights streamed once, both examples)
        def wchain(w_d, rhs, tag):
            ps4 = psm.tile([128, DC, BPC, 2, NCAND], f32, tag="sm", name="ps4")
            for dci in range(DC):
                wt = wstg.tile([128, D], bf16, tag="wt", name="wt")
                nc.gpsimd.dma_start(wt[:], w_d[dci * 128:(dci + 1) * 128, :])
                for dco in range(DC):
                    # one global start per psum tile: a later start=True would
                    # clobber sibling regions' accumulation state in the bank
                    nc.tensor.matmul(ps4[:, dco, :, :, :],
                                     wt[:, dco * 128:(dco + 1) * 128],
                                     rhs[:, dci, :, :, :],
                                     start=(dci == 0 and dco == 0),
                                     stop=(dci == DC - 1),
                                     skip_group_check=True)
            qf = sm.tile([128, DC, BPC, NCAND], f32, tag=tag + "f", name=tag + "f")
            nc.vector.tensor_copy(qf[:], ps4[:, :, :, 0, :])
            nc.vector.tensor_add(qf[:], qf[:], ps4[:, :, :, 1, :])
            pair = sm.tile([128, DC, BPC, 2, NCAND], bf16, tag=tag, name=tag)
            nc.vector.tensor_copy(pair[:, :, :, 0, :], qf[:])
            hf = sm.tile([128, DC, BPC, NCAND], f32, tag=tag + "h", name=tag + "h")
            nc.vector.tensor_copy(hf[:], pair[:, :, :, 0, :])
            nc.vector.tensor_sub(qf[:], qf[:], hf[:])
            nc.vector.tensor_copy(pair[:, :, :, 1, :], qf[:])
            return pair

        hrow_flat = hrow_d.rearrange("b l d -> (b l) d")
        wn8_sb = []
        for b in range(BPC):
            t8v = sm.tile([1, 8], bf16, tag=f"t8v{b}", name="t8v")
            t8p = sm.tile([1, 8], u32, tag=f"t8p{b}", name="t8p")
            nc.vector.max(t8v[:], fL_sb[b][:])
            nc.vector.max_index(t8p[:], t8v[:], fL_sb[b][:])
            t8pf = sm.tile([1, 8], f32, tag=f"t8pf{b}", name="t8pf")
            nc.vector.tensor_copy(t8pf[:], t8p[:])
            nc.vector.tensor_scalar_add(t8pf[:], t8pf[:], float(b * L))
            t8pi = sm.tile([1, 8], u32, tag=f"t8pi{b}", name="t8pi")
            nc.vector.tensor_copy(t8pi[:], t8pf[:])
            idx8 = sm.tile([NCAND, 1], u32, tag=f"idx8{b}", name="idx8")
            nc.sync.dma_start(idx8[:, 0:1], t8pi[0:1, :])
            rows = sm.tile([NCAND, D], f32, tag="rows", name="rows")
            nc.gpsimd.indirect_dma_start(
                out=rows[:], out_offset=None, in_=hrow_flat,
                in_offset=bass.IndirectOffsetOnAxis(ap=idx8[:, 0:1], axis=0))
            # exact fp32 logits for the 8 candidates (f32 products so the
            # reduce is fp32-exact; tensor_tensor_reduce is avoided — it
            # crashes the device on this runtime)
            prod = sm.tile([NCAND, D], f32, tag="ek", name="prod")
            e8 = sm.tile([NCAND, 1], f32, tag=f"e8{b}", name="e8")
            nc.vector.tensor_mul(prod[:], rows[:], wsf8[:])
            nc.vector.tensor_reduce(e8[:], prod[:], AX.X, ALU.add)
            e8r = sm.tile([1, 8], f32, tag=f"e8r{b}", name="e8r")
            nc.sync.dma_start(e8r[0:1, :], e8[:])
            s8 = sm.tile([1, 8], f32, tag=f"s8{b}", name="s8")
            nc.vector.max(s8[:], e8r[:])
            thr = sm.tile([1, 1], f32, tag=f"thr{b}", name="thr")
            nc.vector.tensor_add(thr[:], s8[0:1, K - 1:K], s8[0:1, K:K + 1])
            nc.vector.tensor_scalar_mul(thr[:], thr[:], 0.5)
            msk = sm.tile([1, 8], f32, tag=f"msk{b}", name="msk")
            nc.vector.tensor_scalar(msk[:], e8r[:], thr[:], None, ALU.is_gt)
            negmx = sm.tile([1, 1], f32, tag=f"negmx{b}", name="negmx")
            nc.vector.tensor_scalar_mul(negmx[:], s8[0:1, 0:1], -1.0)
            ew = sm.tile([1, 8], f32, tag=f"ew{b}", name="ew")
            nc.scalar.activation(ew[:], e8r[:], AF.Exp, bias=negmx[:], scale=1.0)
            w8m = sm.tile([1, 8], f32, tag=f"w8m{b}", name="w8m")
            nc.vector.tensor_mul(w8m[:], ew[:], msk[:])
            sw = sm.tile([1, 1], f32, tag=f"sw{b}", name="sw")
            nc.vector.tensor_reduce(sw[:], w8m[:], AX.X, ALU.add)
            rsw = sm.tile([1, 1], f32, tag=f"rsw{b}", name="rsw")
            nc.vector.reciprocal(rsw[:], sw[:])
            wn = sm.tile([1, 8], f32, tag=f"wn{b}", name="wn")
            nc.vector.tensor_scalar_mul(wn[:], w8m[:], rsw[:])
            wn8 = sm.tile([NCAND, 1], f32, tag=f"wn8_{b}", name=f"wn8_{b}")
            wn8_sb.append(wn8)
            nc.sync.dma_start(wn8[:, 0:1], wn[0:1, :])

            # S3 for this example: PE transpose of the gathered fp32 rows
            psr = psm.tile([128, DC, NCAND], f32, tag="sm", name="psr")
            for dc in range(DC):
                nc.tensor.matmul(psr[:, dc, :],
                                 rows[:, dc * 128:(dc + 1) * 128], i8[:],
                                 is_transpose=True, start=True, stop=True,
                                 skip_group_check=True)
            nc.vector.tensor_copy(srhs[:, :, b, 0, :], psr[:])
            nc.vector.tensor_copy(sr_hf[:], srhs[:, :, b, 0, :])
            nc.vector.tensor_sub(sr_lf[:], psr[:], sr_hf[:])
            nc.vector.tensor_copy(srhs[:, :, b, 1, :], sr_lf[:])

        qpair = wchain(wq_d, srhs, "qp")
        ppair = wchain(wkt_d, qpair, "pp")

        # ---- S5 both examples (PE back-to-back), chunk maxes on the fly
        sct_sb, mxc_sb = [], []
        for b in range(BPC):
            sct = sm.tile([NCAND, L], bf16, tag=f"sct{b}", name=f"sct{b}")
            sct_sb.append(sct)
            mxc = sm.tile([NCAND, NJ], f32, tag=f"mxc{b}", name=f"mxc{b}")
            mxc_sb.append(mxc)
            for j in range(NJ):
                ps5 = pbig.tile([16, 512], f32, tag="mm", name="ps5")
                for dc in range(DC):
                    nc.tensor.matmul(ps5[:], ppair[:, dc, b, :, :],
                                     ht_sb[b][:, dc, j * 512:(j + 1) * 512],
                                     start=(dc == 0), stop=(dc == DC - 1))
                # fold hi+lo rows: engines cannot cross partition bases, so
                # stage via scalar copy + DMA partition move, then DVE add
                cp5 = sm.tile([16, 512], f32, tag="cp5", bufs=2, name="cp5")
                nc.scalar.copy(cp5[:], ps5[:])
                cp5b = sm.tile([NCAND, 512], f32, tag="cp5b", bufs=2, name="cp5b")
                nc.sync.dma_start(cp5b[:], cp5[NCAND:16, :])
                nc.vector.tensor_add(sct[:, j * 512:(j + 1) * 512],
                                     cp5[0:NCAND, :], cp5b[:])
                nc.vector.tensor_reduce(mxc[:, j:j + 1],
                                        sct[:, j * 512:(j + 1) * 512],
                                        AX.X, ALU.max)

        # ---- softmax + m broadcast + S6 per example (shared big scratch)
        amix = sm.tile([128, DC, BPC], f32, tag="amix", name="amix")
        for b in range(BPC):
            mx8 = sm.tile([NCAND, 1], f32, tag="mx8", name="mx8")
            nc.vector.tensor_reduce(mx8[:], mxc_sb[b][:], AX.X, ALU.max)
            nbias = sm.tile([NCAND, 1], f32, tag="nbias", name="nbias")
            nc.vector.tensor_scalar_mul(nbias[:], mx8[:], -SCALE)
            ek = sm.tile([NCAND, L], bf16, tag="ek", name="ek")
            z8 = sm.tile([NCAND, 1], f32, tag="z8", name="z8")
            nc.scalar.activation(ek[:], sct_sb[b][:], AF.Exp, bias=nbias[:],
                                 scale=SCALE, accum_out=z8[:])
            rz8 = sm.tile([NCAND, 1], f32, tag="rz8", name="rz8")
            nc.vector.reciprocal(rz8[:], z8[:])
            c8 = sm.tile([NCAND, 1], f32, tag="c8", name="c8")
            nc.vector.tensor_mul(c8[:], wn8_sb[b][:], rz8[:])
            nc.vector.tensor_scalar_mul(ek[:], ek[:], c8[:])   # ek *= c8
            # m as [32, 128] (nat-layout rows), then transpose to [128, 32]
            mt32 = sm.tile([32, 128], bf16, tag="mt32", name="mt32")
            for j in range(NJ):
                pm = pbig.tile([16, 512], f32, tag="mm", name="pm")
                nc.tensor.matmul(pm[0:1, :], ones8[:],
                                 ek[:, j * 512:(j + 1) * 512],
                                 start=True, stop=True)
                m1 = sm.tile([1, 512], bf16, tag="m1", bufs=2, name="m1")
                nc.scalar.copy(m1[:], pm[0:1, :])
                nc.sync.dma_start(mt32[4 * j:4 * j + 4, :], m1[:])
            pt = psm.tile([128, 32], bf16, tag="sm", name="pt")
            nc.tensor.matmul(pt[:], mt32[:], i32[:], is_transpose=True,
                             start=True, stop=True)
            mt = sm.tile([128, 32], bf16, tag="mt", name="mt")
            nc.vector.tensor_copy(mt[:], pt[:])
            # S6: a_mix = sum_l H[l, d] * m[l] on PE, streaming natural H
            ps6 = psm.tile([128, DC, 1], f32, tag="sm", name="ps6")
            NLC = L // 128
            for lc2 in range(NLC // 2):
                nat = natp.tile([128, 2, D], bf16, tag="nat", name="nat")
                nc.gpsimd.dma_start(
                    nat[:],
                    nat_d[b, lc2 * 256:(lc2 + 1) * 256, :]
                    .rearrange("(i p) d -> p i d", p=128))
                for i in range(2):
                    lc = lc2 * 2 + i
                    for dc in range(DC):
                        nc.tensor.matmul(ps6[:, dc, :],
                                         nat[:, i, dc * 128:(dc + 1) * 128],
                                         mt[:, lc:lc + 1],
                                         start=(lc == 0 and dc == 0),
                                         stop=(lc == NLC - 1),
                                         skip_group_check=True)
            nc.scalar.copy(amix[:, :, b:b + 1], ps6[:])

        # ---- split helper [128, DC, BPC] f32 -> [128, 2, DC, BPC] bf16
        def split2(src, tag):
            pair = sm.tile([128, DC, 2, BPC], bf16, tag=tag, name=tag)
            nc.vector.tensor_copy(pair[:, :, 0, :], src[:])
            hf = sm.tile([128, DC, BPC], f32, tag=tag + "h", name=tag + "h")
            nc.vector.tensor_copy(hf[:], pair[:, :, 0, :])
            nc.vector.tensor_sub(hf[:], src[:], hf[:])
            nc.vector.tensor_copy(pair[:, :, 1, :], hf[:])
            return pair

        arhs = split2(amix, "arhs")

        # ---- S7: c_mix (wv), g (wct), shared weight streams for both examples
        def wchain2(w_d, rhs, tag):
            ps7 = psm.tile([128, DC, 2, BPC], f32, tag="sm", name="ps7")
            for dci in range(DC):
                wt = wstg.tile([128, D], bf16, tag="wt", name="wt")
                nc.gpsimd.dma_start(wt[:], w_d[dci * 128:(dci + 1) * 128, :])
                for dco in range(DC):
                    nc.tensor.matmul(ps7[:, dco, :, :],
                                     wt[:, dco * 128:(dco + 1) * 128],
                                     rhs[:, dci, :, :],
                                     start=(dci == 0 and dco == 0),
                                     stop=(dci == DC - 1),
                                     skip_group_check=True)
            outf = sm.tile([128, DC, BPC], f32, tag=tag, name=tag)
            nc.vector.tensor_copy(outf[:], ps7[:, :, 0, :])
            nc.vector.tensor_add(outf[:], outf[:], ps7[:, :, 1, :])
            return outf

        cmix = wchain2(wv_d, arhs, "cm")
        crhs = split2(cmix, "crhs")
        g_f = wchain2(wct_d, crhs, "gg")
        gs = sm.tile([128, DC, BPC], f32, tag="gs", name="gs")
        nc.vector.tensor_scalar_mul(gs[:], g_f[:], SCALE)
        grhs = split2(gs, "grhs")

        # ---- S8: end logits from resident ht
        for b in range(BPC):
            for j in range(NJ):
                ps8 = pbig.tile([16, 512], f32, tag="mm", name="ps8")
                for dc in range(DC):
                    nc.tensor.matmul(ps8[0:2, :], grhs[:, dc, :, b],
                                     ht_sb[b][:, dc, j * 512:(j + 1) * 512],
                                     start=(dc == 0), stop=(dc == DC - 1))
                cp8 = sm.tile([2, 512], f32, tag="cp5", bufs=2, name="cp8")
                nc.scalar.copy(cp8[:], ps8[0:2, :])
                cp8b = sm.tile([1, 512], f32, tag="cp5b", bufs=2, name="cp8b")
                nc.sync.dma_start(cp8b[:], cp8[1:2, :])
                etmp = sm.tile([1, 512], f32, tag="ltmp", bufs=2, name="etmp")
                nc.vector.tensor_add(etmp[:], cp8[0:1, :], cp8b[:])
                nc.sync.dma_start(el_d[b:b + 1, j * 512:(j + 1) * 512], etmp[:])

    if os.environ.get("KERNEL_BUILD_INFO"):
        print(f"[kernel] sbuf remaining: {nc.sbuf_bytes_remaining} bytes")
    nc.compile()
    _NC_CACHE["nc"] = nc
    return nc


def _np_reference(H, attention_mask, w_start, b_start, w_q, b_q, w_k, b_k,
                  w_v, b_v, w_cmp, b_cmp):
    NEG = -1e9
    H = H.astype(np.float32)
    pad = attention_mask == 0
    sl = (H @ w_start + b_start)[..., 0]
    sl = np.where(pad, NEG, sl)
    x = sl - sl.max(-1, keepdims=True)
    e = np.exp(x); sp = e / e.sum(-1, keepdims=True)
    idx = np.argsort(-sp, axis=-1, kind="stable")[:, :K]
    tp = np.take_along_axis(sp, idx, axis=1)
    sr = np.take_along_axis(H, idx[..., None], axis=1)
    Q = sr @ w_q + b_q
    K_ = H @ w_k + b_k
    V = H @ w_v + b_v
    sc = np.einsum('bkd,bld->bkl', Q, K_) * SCALE
    sc = np.where(pad[:, None, :], NEG, sc)
    sc = sc - sc.max(-1, keepdims=True)
    a = np.exp(sc); a = a / a.sum(-1, keepdims=True)
    ctx_ = np.einsum('bkl,bld->bkd', a, V)
    tcmp = H @ w_cmp + b_cmp
    es = np.einsum('bkd,bld->bkl', ctx_, tcmp) * SCALE
    es = np.where(pad[:, None, :], NEG, es)
    w = tp / (tp.sum(-1, keepdims=True) + 1e-9)
    el = np.einsum('bk,bkl->bl', w, es)
    el = np.where(pad, NEG, el)
    return sl, el


def kernel(**inputs):
    H = np.asarray(inputs["H"], np.float32)
    mask = np.asarray(inputs["attention_mask"])
    b_start = np.asarray(inputs["b_start"], np.float32)
    biases_zero = all(np.all(np.asarray(inputs[n]) == 0)
                      for n in ["b_q", "b_k", "b_v", "b_cmp"])
    if not bool((mask == 1).all()) or not biases_zero:
        sl, el = _np_reference(**{k: np.asarray(v) for k, v in inputs.items()})
        return np.asarray(sl, np.float32), np.asarray(el, np.float32)

    w_start = np.asarray(inputs["w_start"], np.float32)
    w_q = np.asarray(inputs["w_q"], np.float32)
    w_k = np.asarray(inputs["w_k"], np.float32)
    w_v = np.asarray(inputs["w_v"], np.float32)
    w_cmp = np.asarray(inputs["w_cmp"], np.float32)

    hi = H.astype(bfnp)
    ht = np.ascontiguousarray(hi.transpose(0, 2, 1)).reshape(B, DC, 128, L)
    wsb = w_start[:, 0].astype(bfnp).reshape(DC, 128, 1)
    wsf8 = np.ascontiguousarray(
        np.broadcast_to(w_start[:, 0], (NCAND, D))).astype(np.float32)

    nc = _build_nc()
    in_maps = []
    for c in range(NCORES):
        s = slice(c * BPC, (c + 1) * BPC)
        in_maps.append({
            "ht": ht[s], "hrow": H[s], "nat": hi[s],
            "wq": w_q.astype(bfnp),
            "wkt": np.ascontiguousarray(w_k.T).astype(bfnp),
            "wv": w_v.astype(bfnp),
            "wct": np.ascontiguousarray(w_cmp.T).astype(bfnp),
            "wsb": wsb, "wsf8": wsf8,
            "i8": np.eye(NCAND, dtype=np.float32),
            "ones8": np.ones((NCAND, 1), bfnp),
            "i32": np.eye(32, dtype=np.float32).astype(bfnp),
        })
    import time as _time
    _t0 = _time.time()
    kw = {}
    if os.environ.get("KERNEL_PROFILE"):
        kw = dict(trace=True,
                  tmpdir=os.environ.get("KERNEL_PROFILE_DIR") or None,
                  trace_cores=[int(x) for x in
                               os.environ.get("KERNEL_TRACE_CORES", "0").split(",")])
    res = run_bass_kernel_spmd(nc, in_maps, core_ids=list(range(NCORES)), **kw)
    LAST["res"] = res
    if os.environ.get("KERNEL_TIME"):
        print(f"[kernel] device dispatch+exec wall: {_time.time() - _t0:.3f}s")
    sl = np.concatenate([r["sl"] for r in res.results], 0) + b_start[0]
    el = np.concatenate([r["el"] for r in res.results], 0)
    return sl.astype(np.float32), el.astype(np.float32)
